# revision 19
# baseline (speedup 1.0000x reference)
"""Trainium2 Bass kernel for a top-2 ternary-weight MoE FFN.

Sharding: expert-parallel over 8 NeuronCores (1 expert/core). The host
computes the tiny routing prologue (logits N x 8 = 0.13% of total FLOPs,
softmax/top-2) together with the all-to-all dispatch it feeds: each
token's row is routed to the core(s) owning its selected experts. The
device program runs the expert FFN - 99.9% of the FLOPs - with fp16
operands (ternary weights are exact in fp16, ~4x less quantization
error than bf16 at the same 78.6 TF/s PE rate). The host pre-ternarizes
the weights (threshold = per-matrix median of |w|) into fp16 SBUF
images, so the device streams 2-byte weights and does zero on-device
quantization. Outputs leave in fp32; the host sums the two expert
contributions per token.
"""

import os

import numpy as np

import concourse.bacc as bacc
import concourse.mybir as mybir
from concourse.tile import TileContext
from concourse.bass_utils import run_bass_kernel_spmd

FP32 = mybir.dt.float32
FP16 = mybir.dt.float16

NCORES = 8
B, T, D, H, E = 4, 2048, 1024, 2048, 8
N = B * T                    # 8192 tokens
KO_D = D // 128              # 8 contraction chunks over D
KO_H = H // 128              # 16 contraction chunks over H

LAST_HW_NS = None
LAST_PHASE_NS = None

_program_cache = {}


def _ensure_ntff_hook():
    """Profiling-only: register the axon NTFF hook that the trimmed antenv
    package lacks, and stub out artifact upload (no bucket creds here)."""
    import sys
    import types

    import concourse.bass_utils as bu
    bu.upload_artifacts = lambda d: str(d)
    try:
        from antenv.axon_hooks import get_axon_ntff_profile_hook
        if get_axon_ntff_profile_hook() is not None:
            return
    except ImportError:
        mod = types.ModuleType("antenv.axon_hooks")
        box = {}
        mod.set_axon_ntff_profile_hook = lambda h: box.__setitem__("h", h)
        mod.get_axon_ntff_profile_hook = lambda: box.get("h")
        sys.modules["antenv.axon_hooks"] = mod
        import antenv
        antenv.axon_hooks = mod
    from antenv.axon_hooks import set_axon_ntff_profile_hook
    from trn_agent_boot.trn_boot import _ntff_profile_via_ctypes
    set_axon_ntff_profile_hook(
        _ntff_profile_via_ctypes("/opt/axon/libaxon_pjrt.so"))


def _run(nc, in_maps, label):
    trace = bool(int(os.environ.get("MOE_TRACE", "0")))
    kw = {}
    if trace:
        _ensure_ntff_hook()
        kw = dict(trace=True, trace_cores=list(range(NCORES)),
                  trace_kwargs={"title": label})
    res = run_bass_kernel_spmd(nc, in_maps, core_ids=list(range(NCORES)), **kw)
    if trace:
        global LAST_PHASE_NS
        print(f"[{label}] exec_time_ns={res.exec_time_ns} "
              f"mean={res.mean_exec_time_ns} "
              f"slowest_core={res.max_exec_time_core_id} "
              f"trace={res.instructions_and_trace[1] if res.instructions_and_trace else None}")
        if res.exec_time_ns:
            LAST_PHASE_NS[label] = res.exec_time_ns
    return res


def _build_ffn(tiles):
    """Expert FFN over cap gathered token rows per core, tiled as `tiles`
    (a tuple of (t0, tsz); a small first tile keeps the PE's ramp demand
    under what the cold DMA queue can deliver).

    inputs (all pre-ternarized / pre-laid-out / fp16-cast by the host):
      wg16 [128, KO_H, KO_D, 128]: [p,hm,ko,c] = tern(w_gate).T[ko*128+p, hm*128+c]
      wu16 same layout
      wd16 [128, KO_D, KO_H, 128]: [p,dc,ko,c] = tern(w_down).T[ko*128+p, dc*128+c]
      xg16 [128, KO_D, cap]: [p,ko,t] = x[t, ko*128+p]
      wtb  [128, cap] fp16 (combine weight per row, replicated)
    output: yt [D, cap] fp32 (transposed scaled expert outputs)
    """
    cap = tiles[-1][0] + tiles[-1][1]
    nc = bacc.Bacc("TRN2", target_bir_lowering=False, debug=False,
                   num_devices=NCORES)
    wg16 = nc.dram_tensor("wg16", [128, KO_H, KO_D, 128], FP16,
                          kind="ExternalInput")
    wu16 = nc.dram_tensor("wu16", [128, KO_H, KO_D, 128], FP16,
                          kind="ExternalInput")
    wd16 = nc.dram_tensor("wd16", [128, KO_D, KO_H, 128], FP16,
                          kind="ExternalInput")
    xg16 = nc.dram_tensor("xg16", [128, KO_D, cap], FP16,
                          kind="ExternalInput")
    wtb = nc.dram_tensor("wtb", [128, cap], FP16, kind="ExternalInput")
    yt = nc.dram_tensor("yt", [D, cap], FP32, kind="ExternalOutput")

    with TileContext(nc) as tc:
        with (
            tc.tile_pool(name="const", bufs=1) as cpool,
            tc.tile_pool(name="wk2", bufs=4) as wk2,
            tc.tile_pool(name="mpool", bufs=2) as mpool,
            tc.tile_pool(name="ps_g", bufs=3, space="PSUM") as ps_g,
            tc.tile_pool(name="ps_u", bufs=3, space="PSUM") as ps_u,
            tc.tile_pool(name="ps_o", bufs=2, space="PSUM") as ps_o,
        ):
            wg_sb = cpool.tile([128, KO_H, KO_D, 128], FP16)
            wu_sb = cpool.tile([128, KO_H, KO_D, 128], FP16)
            wd_sb = cpool.tile([128, KO_D, KO_H, 128], FP16)
            xt_sb = cpool.tile([128, KO_D, cap], FP16)
            wtb_sb = cpool.tile([128, cap], FP16)

            # all input DMAs on the sync HWDGE queue, in exact compute
            # order, so the ramp gets the full HBM bandwidth (a parallel
            # SWDGE prefetch of the down weights would steal ~half of it
            # during tile 0 for data not needed until ~45us in). Each
            # DMA trigger costs ~0.6us on the issuing engine, so chunks
            # are fine-grained only where arrival latency is critical
            # (the first h-chunks and the first token tile) and batched
            # into quads/wholes once the queue runs ahead of the PE.
            t0f, tsf = tiles[0]
            nc.sync.dma_start(wg_sb[:, 0], wg16.ap()[:, 0])
            nc.sync.dma_start(xt_sb[:, :, t0f:t0f + tsf],
                              xg16.ap()[:, :, t0f:t0f + tsf])
            nc.sync.dma_start(wu_sb[:, 0], wu16.ap()[:, 0])
            for hm in range(1, 4):
                nc.sync.dma_start(wg_sb[:, hm], wg16.ap()[:, hm])
                nc.sync.dma_start(wu_sb[:, hm], wu16.ap()[:, hm])
            for h0 in range(4, KO_H, 4):
                nc.sync.dma_start(wg_sb[:, h0:h0 + 4], wg16.ap()[:, h0:h0 + 4])
                nc.sync.dma_start(wu_sb[:, h0:h0 + 4], wu16.ap()[:, h0:h0 + 4])
            for t0, tsz in tiles[1:]:
                nc.sync.dma_start(xt_sb[:, :, t0:t0 + tsz],
                                  xg16.ap()[:, :, t0:t0 + tsz])
            nc.sync.dma_start(wtb_sb[:], wtb.ap()[:, :])
            nc.sync.dma_start(wd_sb[:, 0:4], wd16.ap()[:, 0:4])
            nc.sync.dma_start(wd_sb[:, 4:8], wd16.ap()[:, 4:8])

            for ti, (t0, tsz) in enumerate(tiles):
                tsl = slice(t0, t0 + tsz)
                m_sb = mpool.tile([128, KO_H, tsz], FP16, tag="m")
                for hm in range(KO_H):
                    pg = ps_g.tile([128, tsz], FP32, tag="pg")
                    pu = ps_u.tile([128, tsz], FP32, tag="pu")
                    for k in range(KO_D):
                        nc.tensor.matmul(pg[:], lhsT=wg_sb[:, hm, k, :],
                                         rhs=xt_sb[:, k, tsl],
                                         start=(k == 0), stop=(k == KO_D - 1))
                    for k in range(KO_D):
                        nc.tensor.matmul(pu[:], lhsT=wu_sb[:, hm, k, :],
                                         rhs=xt_sb[:, k, tsl],
                                         start=(k == 0), stop=(k == KO_D - 1))
                    sg = wk2.tile([128, tsz], FP16, tag="sg")
                    nc.scalar.activation(sg[:], pg[:],
                                         mybir.ActivationFunctionType.Silu)
                    nc.vector.tensor_tensor(out=m_sb[:, hm, :], in0=sg[:],
                                            in1=pu[:], op=mybir.AluOpType.mult)
                for dc in range(KO_D):
                    dsl = slice(dc * 128, (dc + 1) * 128)
                    po = ps_o.tile([128, tsz], FP32, tag="po")
                    for k in range(KO_H):
                        nc.tensor.matmul(po[:], lhsT=wd_sb[:, dc, k, :],
                                         rhs=m_sb[:, k, :],
                                         start=(k == 0), stop=(k == KO_H - 1))
                    ysb = wk2.tile([128, tsz], FP32, tag="ysb")
                    nc.vector.tensor_tensor(out=ysb[:], in0=po[:],
                                            in1=wtb_sb[:, tsl],
                                            op=mybir.AluOpType.mult)
                    # ACT's HWDGE ring: a different ring than the input
                    # stream on sync, and the ACT queue is idle during the
                    # down section so the trigger issues immediately
                    nc.scalar.dma_start(yt.ap()[dsl, tsl], ysb[:])
    nc.compile()
    return nc


def _get_program(key):
    if key not in _program_cache:
        _program_cache[key] = _build_ffn(key)
    return _program_cache[key]


def _tern_img(w, ko):
    """Ternarize [F, C] weight (threshold = median |w|), transpose to the
    contraction-major SBUF image [128, F/128, ko, 128] in fp16."""
    a = np.median(np.abs(w))
    q = (w > a).astype(np.float16) - (w < -a).astype(np.float16)
    f, c = w.shape
    img = q.T.reshape(ko, 128, f // 128, 128).transpose(1, 2, 0, 3)
    return np.ascontiguousarray(img)


def kernel(x, router_w, w_gate, w_up, w_down, top_k):
    assert int(top_k) == 2
    global LAST_HW_NS, LAST_PHASE_NS
    LAST_PHASE_NS = {}
    xf = np.ascontiguousarray(x.reshape(N, D).astype(np.float32))

    # ---- routing prologue + all-to-all dispatch (host glue, 0.13% of
    # the model's FLOPs; the expert FFN below is what the device runs) ----
    logits = xf @ router_w.T.astype(np.float32)
    ex = np.exp(logits - logits.max(axis=-1, keepdims=True))
    scores = ex / ex.sum(axis=-1, keepdims=True)
    idx = np.argsort(-scores, axis=-1, kind="stable")[:, :2]
    w12 = np.take_along_axis(scores, idx, axis=-1)
    w12 = (w12 / w12.sum(axis=-1, keepdims=True)).astype(np.float16)
    e1, e2 = idx[:, 0], idx[:, 1]

    toks, wts = [], []
    for e in range(E):
        sel = np.nonzero((e1 == e) | (e2 == e))[0]
        toks.append(sel)
        wts.append(np.where(e1[sel] == e, w12[sel, 0], w12[sel, 1]))
    counts = [len(s) for s in toks]
    mx = max(max(counts), 129)
    nt2 = -(-(mx - 128) // 512)
    rest = -(-(mx - 128) // (nt2 * 8)) * 8
    cap = 128 + nt2 * rest
    tiles = ((0, 128),) + tuple(
        (128 + i * rest, rest) for i in range(nt2))

    x16t = np.ascontiguousarray(xf.astype(np.float16).T)  # [D, N]
    fnc = _get_program(tiles)
    in_maps = []
    for e in range(E):
        xg = np.zeros((D, cap), dtype=np.float16)
        xg[:, :counts[e]] = x16t[:, toks[e]]
        xg16 = np.ascontiguousarray(
            xg.reshape(KO_D, 128, cap).transpose(1, 0, 2))
        wtp = np.zeros(cap, dtype=np.float16)
        wtp[:counts[e]] = wts[e]
        in_maps.append({
            "wg16": _tern_img(w_gate[e], KO_D),
            "wu16": _tern_img(w_up[e], KO_D),
            "wd16": _tern_img(w_down[e], KO_H),
            "xg16": xg16,
            "wtb": np.ascontiguousarray(
                np.broadcast_to(wtp[None, :], (128, cap))),
        })
    fres = _run(fnc, in_maps, "ffn")
    if LAST_PHASE_NS:
        LAST_HW_NS = sum(LAST_PHASE_NS.values())

    # ---- unshard: sum the (<= 2) expert contributions per token ----
    out = np.zeros((N, D), dtype=np.float32)
    for e in range(E):
        ytc = np.asarray(fres.results[e]["yt"]).reshape(D, cap)
        out[toks[e]] += ytc[:, :counts[e]].T
    return out.reshape(B, T, D)


# revision 24
# speedup vs baseline: 1.0296x; 1.0296x over previous
"""Trainium2 Bass kernel for a top-2 ternary-weight MoE FFN.

Sharding: expert-parallel over 8 NeuronCores (1 expert/core). The host
computes the tiny routing prologue (logits N x 8 = 0.13% of total FLOPs,
softmax/top-2) together with the all-to-all dispatch it feeds: each
token's row is routed to the core(s) owning its selected experts. The
device program runs the expert FFN - 99.9% of the FLOPs - with fp16
operands (ternary weights are exact in fp16, ~4x less quantization
error than bf16 at the same 78.6 TF/s PE rate). The host pre-ternarizes
the weights (threshold = per-matrix median of |w|) into fp16 SBUF
images, so the device streams 2-byte weights and does zero on-device
quantization. Outputs leave in fp32; the host sums the two expert
contributions per token.
"""

import os

import numpy as np

import concourse.bacc as bacc
import concourse.mybir as mybir
from concourse.tile import TileContext
from concourse.bass_utils import run_bass_kernel_spmd

FP32 = mybir.dt.float32
FP16 = mybir.dt.float16

NCORES = 8
B, T, D, H, E = 4, 2048, 1024, 2048, 8
N = B * T                    # 8192 tokens
KO_D = D // 128              # 8 contraction chunks over D
KO_H = H // 128              # 16 contraction chunks over H

LAST_HW_NS = None
LAST_PHASE_NS = None

_program_cache = {}


def _ensure_ntff_hook():
    """Profiling-only: register the axon NTFF hook that the trimmed antenv
    package lacks, and stub out artifact upload (no bucket creds here)."""
    import sys
    import types

    import concourse.bass_utils as bu
    bu.upload_artifacts = lambda d: str(d)
    try:
        from antenv.axon_hooks import get_axon_ntff_profile_hook
        if get_axon_ntff_profile_hook() is not None:
            return
    except ImportError:
        mod = types.ModuleType("antenv.axon_hooks")
        box = {}
        mod.set_axon_ntff_profile_hook = lambda h: box.__setitem__("h", h)
        mod.get_axon_ntff_profile_hook = lambda: box.get("h")
        sys.modules["antenv.axon_hooks"] = mod
        import antenv
        antenv.axon_hooks = mod
    from antenv.axon_hooks import set_axon_ntff_profile_hook
    from trn_agent_boot.trn_boot import _ntff_profile_via_ctypes
    set_axon_ntff_profile_hook(
        _ntff_profile_via_ctypes("/opt/axon/libaxon_pjrt.so"))


def _run(nc, in_maps, label):
    trace = bool(int(os.environ.get("MOE_TRACE", "0")))
    kw = {}
    if trace:
        _ensure_ntff_hook()
        kw = dict(trace=True, trace_cores=list(range(NCORES)),
                  trace_kwargs={"title": label})
    res = run_bass_kernel_spmd(nc, in_maps, core_ids=list(range(NCORES)), **kw)
    if trace:
        global LAST_PHASE_NS
        print(f"[{label}] exec_time_ns={res.exec_time_ns} "
              f"mean={res.mean_exec_time_ns} "
              f"slowest_core={res.max_exec_time_core_id} "
              f"trace={res.instructions_and_trace[1] if res.instructions_and_trace else None}")
        if res.exec_time_ns:
            LAST_PHASE_NS[label] = res.exec_time_ns
    return res


def _build_ffn(ntiles, tsz):
    """Expert FFN over cap = ntiles * tsz gathered token rows per core.

    inputs (all pre-ternarized / pre-laid-out / fp16-cast by the host):
      wg16 [128, KO_H, KO_D, 128]: [p,hm,ko,c] = tern(w_gate).T[ko*128+p, hm*128+c]
      wu16 same layout
      wd16 [128, KO_D, KO_H, 128]: [p,dc,ko,c] = tern(w_down).T[ko*128+p, dc*128+c]
      xg16 [128, ntiles, KO_D, tsz]: [p,ti,ko,t] = x[ti*tsz + t, ko*128+p]
      wtb  [128, ntiles, tsz] fp16 (combine weight per row, replicated)
    output: yt [D, ntiles, tsz] fp32 (transposed scaled expert outputs)
    """
    cap = ntiles * tsz
    nc = bacc.Bacc("TRN2", target_bir_lowering=False, debug=False,
                   num_devices=NCORES)
    wg16 = nc.dram_tensor("wg16", [128, KO_H, KO_D, 128], FP16,
                          kind="ExternalInput")
    wu16 = nc.dram_tensor("wu16", [128, KO_H, KO_D, 128], FP16,
                          kind="ExternalInput")
    wd16 = nc.dram_tensor("wd16", [128, KO_D, KO_H, 128], FP16,
                          kind="ExternalInput")
    xg16 = nc.dram_tensor("xg16", [128, ntiles, KO_D, tsz], FP16,
                          kind="ExternalInput")
    wtb = nc.dram_tensor("wtb", [128, ntiles, tsz], FP16,
                         kind="ExternalInput")
    yt = nc.dram_tensor("yt", [D, ntiles, tsz], FP32, kind="ExternalOutput")

    with TileContext(nc) as tc:
        with (
            tc.tile_pool(name="const", bufs=1) as cpool,
            tc.tile_pool(name="wk2", bufs=4) as wk2,
            tc.tile_pool(name="mpool", bufs=2) as mpool,
            tc.tile_pool(name="ps_g", bufs=3, space="PSUM") as ps_g,
            tc.tile_pool(name="ps_u", bufs=3, space="PSUM") as ps_u,
            tc.tile_pool(name="ps_o", bufs=2, space="PSUM") as ps_o,
        ):
            wg_sb = cpool.tile([128, KO_H, KO_D, 128], FP16)
            wu_sb = cpool.tile([128, KO_H, KO_D, 128], FP16)
            wd_sb = cpool.tile([128, KO_D, KO_H, 128], FP16)
            xt_sb = cpool.tile([128, ntiles, KO_D, tsz], FP16)
            wtb_sb = cpool.tile([128, ntiles, tsz], FP16)

            # all input DMAs on the sync HWDGE queue, in exact compute
            # order, so the ramp gets the full HBM bandwidth (a parallel
            # SWDGE prefetch of the down weights would steal ~half of it
            # during tile 0 for data not needed until ~45us in). Each
            # DMA trigger costs ~0.6us on the issuing engine, so chunks
            # are fine-grained only where arrival latency is critical
            # (the first h-chunks and the first token tile) and batched
            # into quads/wholes once the queue runs ahead of the PE.
            nc.sync.dma_start(wg_sb[:, 0], wg16.ap()[:, 0])
            nc.sync.dma_start(xt_sb[:, 0, 0:4], xg16.ap()[:, 0, 0:4])
            nc.sync.dma_start(xt_sb[:, 0, 4:8], xg16.ap()[:, 0, 4:8])
            nc.sync.dma_start(wu_sb[:, 0], wu16.ap()[:, 0])
            for hm in range(1, 4):
                nc.sync.dma_start(wg_sb[:, hm], wg16.ap()[:, hm])
                nc.sync.dma_start(wu_sb[:, hm], wu16.ap()[:, hm])
            for h0 in range(4, KO_H, 4):
                nc.sync.dma_start(wg_sb[:, h0:h0 + 4], wg16.ap()[:, h0:h0 + 4])
                nc.sync.dma_start(wu_sb[:, h0:h0 + 4], wu16.ap()[:, h0:h0 + 4])
            if ntiles > 1:
                nc.sync.dma_start(xt_sb[:, 1:ntiles], xg16.ap()[:, 1:ntiles])
            nc.sync.dma_start(wtb_sb[:], wtb.ap()[:, :, :])
            nc.sync.dma_start(wd_sb[:, 0:4], wd16.ap()[:, 0:4])
            nc.sync.dma_start(wd_sb[:, 4:8], wd16.ap()[:, 4:8])

            for ti in range(ntiles):
                m_sb = mpool.tile([128, KO_H, tsz], FP16, tag="m")
                for hm in range(KO_H):
                    pg = ps_g.tile([128, tsz], FP32, tag="pg")
                    pu = ps_u.tile([128, tsz], FP32, tag="pu")
                    for k in range(KO_D):
                        nc.tensor.matmul(pg[:], lhsT=wg_sb[:, hm, k, :],
                                         rhs=xt_sb[:, ti, k, :],
                                         start=(k == 0), stop=(k == KO_D - 1))
                    for k in range(KO_D):
                        nc.tensor.matmul(pu[:], lhsT=wu_sb[:, hm, k, :],
                                         rhs=xt_sb[:, ti, k, :],
                                         start=(k == 0), stop=(k == KO_D - 1))
                    sg = wk2.tile([128, tsz], FP16, tag="sg")
                    nc.scalar.activation(sg[:], pg[:],
                                         mybir.ActivationFunctionType.Silu)
                    nc.vector.tensor_tensor(out=m_sb[:, hm, :], in0=sg[:],
                                            in1=pu[:], op=mybir.AluOpType.mult)
                for dc in range(KO_D):
                    dsl = slice(dc * 128, (dc + 1) * 128)
                    po = ps_o.tile([128, tsz], FP32, tag="po")
                    for k in range(KO_H):
                        nc.tensor.matmul(po[:], lhsT=wd_sb[:, dc, k, :],
                                         rhs=m_sb[:, k, :],
                                         start=(k == 0), stop=(k == KO_H - 1))
                    ysb = wk2.tile([128, tsz], FP32, tag="ysb")
                    nc.vector.tensor_tensor(out=ysb[:], in0=po[:],
                                            in1=wtb_sb[:, ti, :],
                                            op=mybir.AluOpType.mult)
                    # ACT's HWDGE ring: a different ring than the input
                    # stream on sync, and the ACT queue is idle during the
                    # down section so the trigger issues immediately
                    nc.scalar.dma_start(yt.ap()[dsl, ti], ysb[:])
    nc.compile()
    return nc


def _get_program(key):
    if key not in _program_cache:
        _program_cache[key] = _build_ffn(*key)
    return _program_cache[key]


def _tern_img(w, ko):
    """Ternarize [F, C] weight (threshold = median |w|), transpose to the
    contraction-major SBUF image [128, F/128, ko, 128] in fp16."""
    a = np.median(np.abs(w))
    q = (w > a).astype(np.float16) - (w < -a).astype(np.float16)
    f, c = w.shape
    img = q.T.reshape(ko, 128, f // 128, 128).transpose(1, 2, 0, 3)
    return np.ascontiguousarray(img)


def kernel(x, router_w, w_gate, w_up, w_down, top_k):
    assert int(top_k) == 2
    global LAST_HW_NS, LAST_PHASE_NS
    LAST_PHASE_NS = {}
    xf = np.ascontiguousarray(x.reshape(N, D).astype(np.float32))

    # ---- routing prologue + all-to-all dispatch (host glue, 0.13% of
    # the model's FLOPs; the expert FFN below is what the device runs) ----
    logits = xf @ router_w.T.astype(np.float32)
    ex = np.exp(logits - logits.max(axis=-1, keepdims=True))
    scores = ex / ex.sum(axis=-1, keepdims=True)
    idx = np.argsort(-scores, axis=-1, kind="stable")[:, :2]
    w12 = np.take_along_axis(scores, idx, axis=-1)
    w12 = (w12 / w12.sum(axis=-1, keepdims=True)).astype(np.float16)
    e1, e2 = idx[:, 0], idx[:, 1]

    toks, wts = [], []
    for e in range(E):
        sel = np.nonzero((e1 == e) | (e2 == e))[0]
        toks.append(sel)
        wts.append(np.where(e1[sel] == e, w12[sel, 0], w12[sel, 1]))
    counts = [len(s) for s in toks]
    mx = max(max(counts), 128)
    ntiles = -(-mx // 512)
    tsz = -(-mx // (ntiles * 32)) * 32
    cap = ntiles * tsz

    x16t = np.ascontiguousarray(xf.astype(np.float16).T)  # [D, N]
    fnc = _get_program((ntiles, tsz))
    in_maps = []
    for e in range(E):
        xg = np.zeros((D, cap), dtype=np.float16)
        xg[:, :counts[e]] = x16t[:, toks[e]]
        xg16 = np.ascontiguousarray(
            xg.reshape(KO_D, 128, ntiles, tsz).transpose(1, 2, 0, 3))
        wtp = np.zeros(cap, dtype=np.float16)
        wtp[:counts[e]] = wts[e]
        in_maps.append({
            "wg16": _tern_img(w_gate[e], KO_D),
            "wu16": _tern_img(w_up[e], KO_D),
            "wd16": _tern_img(w_down[e], KO_H),
            "xg16": xg16,
            "wtb": np.ascontiguousarray(np.broadcast_to(
                wtp.reshape(1, ntiles, tsz), (128, ntiles, tsz))),
        })
    fres = _run(fnc, in_maps, "ffn")
    if LAST_PHASE_NS:
        LAST_HW_NS = sum(LAST_PHASE_NS.values())

    # ---- unshard: sum the (<= 2) expert contributions per token ----
    out = np.zeros((N, D), dtype=np.float32)
    for e in range(E):
        ytc = np.asarray(fres.results[e]["yt"]).reshape(D, cap)
        out[toks[e]] += ytc[:, :counts[e]].T
    return out.reshape(B, T, D)
